# revision 1
# baseline (speedup 1.0000x reference)
"""Trainium2 Bass kernel for nn_DiffusionDynamicInput.

Reference computation (per sample b):
    ctx  = wv_embs[b] + t_emb[b]                       (13, 1024)
    hid  = silu(ctx @ w1 + b1)                         (13, 512)
    wgen = (hid @ w2 + b2).reshape(13, 128, 9)         per-(band) 3x3 filters
    out[d,h,w] = sum_{n,dy,dx} wgen[n,d,(dy,dx)] * x[b,n,h+dy,w+dx]   (SAME pad)
    bias = (ctx @ wb + bb).sum(axis=0)                 (128,)
    out += bias[:, None, None]

Sharding: data-parallel over B=8 across the 8 NeuronCores (one sample per
core). Per core the dynamic conv runs as K=39 fp16 matmuls: partition
q = n*3 + dyi holds the full image of band n shifted by dy (rows stored
258 wide with zero pad columns, so the dx shift is a free-dim offset);
the three dx matmuls accumulate in one PSUM bank. x arrives host-cast
to fp16 and host-padded to 258-wide rows, so the shifted replicas are
three fully-contiguous DMA loads into a resident SBUF image
(132 KB/partition). The hypernetwork runs with fp16 operands (host-cast,
host-permuted weights) and fp32 PSUM. The per-sample bias and the
PSUM->SBUF eviction are fused; output DMAs alternate between the two
HWDGE rings (SP/ACT) since the 33.5 MB/core output write is the
bandwidth bottleneck.
"""

import numpy as np

import concourse.bacc as bacc
import concourse.mybir as mybir
import concourse.tile as tile
from concourse.bass_utils import run_bass_kernel_spmd
from concourse.masks import make_identity

F32 = mybir.dt.float32
F16 = mybir.dt.float16

NB = 13          # bands
HH = WW = 256    # image
DE = 1024        # embed dim
DO = 128         # out channels
NCORES = 8

WPAD = WW + 2    # 258: row layout with a zero column at each end
GRP = 8          # psum banks in flight
OSTROWS = 8      # output rows per staging tile / output DMA (1 MB DMAs)


def _build_bass(repeat: int = 1, ablate: str = ""):
    # Bacc (not plain Bass): its finalize() runs generate_event_semaphores,
    # which splits multi-sem waits that TRN2 instruction structs can't hold.
    # repeat > 1 re-emits the main conv loop (benchmarking: slope between
    # repeat counts isolates device time from dispatch overhead).
    ab = set(ablate.split(",")) if ablate else set()
    nc = bacc.Bacc(target_bir_lowering=False, debug=False)

    # x is host-cast to fp16 and host-padded to 258-wide rows (zero col at
    # each end), so the im2col DMAs are fully contiguous per partition
    x_ext = nc.declare_dram_parameter("x", [NB, HH, WPAD], F16, isOutput=False)
    t_ext = nc.declare_dram_parameter("t_emb", [DE], F32, isOutput=False)
    wv_ext = nc.declare_dram_parameter("wv", [NB, DE], F32, isOutput=False)
    # w1/w2p/wb are host-cast to fp16; w2p/b2p host-permuted so generated
    # filter column c' = p*128 + d
    # w1p[p, k, m*128+s] = w1[k*128+p, m*128+s]; similarly w2p along k;
    # wbp[p, k, d] = wb[k*128+p, d]  (one contiguous DMA per weight)
    w1_ext = nc.declare_dram_parameter("w1p", [128, 8, 4 * DO], F16, isOutput=False)
    b1_ext = nc.declare_dram_parameter("b1", [4 * DO], F32, isOutput=False)
    w2p_ext = nc.declare_dram_parameter("w2pp", [128, 4, DO * 9], F16, isOutput=False)
    b2p_ext = nc.declare_dram_parameter("b2p", [DO * 9], F16, isOutput=False)
    wb_ext = nc.declare_dram_parameter("wbp", [128, 8, DO], F16, isOutput=False)
    bb_ext = nc.declare_dram_parameter("bb", [DO], F32, isOutput=False)
    out_ext = nc.declare_dram_parameter("out", [DO, HH, WW], F32, isOutput=True)

    with tile.TileContext(nc) as tc:
        with (
            tc.tile_pool(name="const", bufs=1) as const_pool,
            tc.tile_pool(name="resident", bufs=1) as res_pool,
            tc.tile_pool(name="hyp", bufs=1) as hyp_pool,
        ):
            # ---------------- hypernetwork (fp16 in / fp32 psum) ------------
            ident = const_pool.tile([128, 128], F32)
            make_identity(nc, ident[:])

            tT = hyp_pool.tile([128, 8], F32)   # t_emb[k*128+p] -> [p, k]
            nc.sync.dma_start(tT[:], t_ext.ap().rearrange("(k p) -> p k", p=128))
            b1T = hyp_pool.tile([128, 4], F32)
            nc.sync.dma_start(b1T[:], b1_ext.ap().rearrange("(m p) -> p m", p=128))
            bbT = hyp_pool.tile([128, 1], F32)
            nc.sync.dma_start(bbT[:], bb_ext.ap().rearrange("(p o) -> p o", o=1))
            b2pT = hyp_pool.tile([1, DO * 9], F16)
            nc.sync.dma_start(b2pT[:], b2p_ext.ap().rearrange("(o c) -> o c", o=1))
            ones1 = const_pool.tile([1, NB], F16)
            nc.vector.memset(ones1[:], 1.0)

            wv_t = hyp_pool.tile([NB, DE], F32)
            nc.sync.dma_start(wv_t[:], wv_ext.ap())

            w1p_t = hyp_pool.tile([128, 8, 4 * DO], F16)
            nc.sync.dma_start(w1p_t[:], w1_ext.ap())
            w2p_t = hyp_pool.tile([128, 4, DO * 9], F16)
            nc.sync.dma_start(w2p_t[:], w2p_ext.ap())
            wbp_t = hyp_pool.tile([128, 8, DO], F16)
            nc.sync.dma_start(wbp_t[:], wb_ext.ap())

            # ctxT[e, k, n] = wv[n, k*128+e] + t[k*128+e]   (fp16)
            ctxT = hyp_pool.tile([128, 8, NB], F16)
            with tc.tile_pool(name="tp_psum", bufs=2, space="PSUM") as tp_psum:
                # warm-up op: absorbs the identity-producer (Pool) semaphore
                # into the PE engine clock so later transposes carry a single
                # wait (the fused LDW struct has one wait slot).
                ps_warm = tp_psum.tile([1, 1], F32, tag="warm", bufs=1)
                nc.tensor.transpose(ps_warm[:], ident[:1, :1], ident[:1, :1])
                for k in range(8):
                    ps = tp_psum.tile([128, NB], F32, tag="tp")
                    nc.tensor.transpose(
                        ps[:], wv_t[:, k * 128:(k + 1) * 128], ident[:NB, :NB]
                    )
                    nc.vector.tensor_scalar_add(ctxT[:, k, :], ps[:], tT[:, k:k + 1])

                # sT[e, k] = sum_n ctxT[e, k, n]   (fp16 for the wb matmul)
                sT32 = hyp_pool.tile([128, 8, 1], F32)
                nc.vector.reduce_sum(sT32[:], ctxT[:], axis=mybir.AxisListType.X)
                sT = hyp_pool.tile([128, 8, 1], F16)
                nc.vector.tensor_copy(sT[:], sT32[:])

                # hidT[s, m, n] = silu(sum_e w1[e, m*128+s] * ctxT[e, n] + b1)
                hidT = hyp_pool.tile([128, 4, NB], F16)
                for m in range(4):
                    ps = tp_psum.tile([128, NB], F32, tag="hid")
                    for k in range(8):
                        nc.tensor.matmul(
                            ps[:], w1p_t[:, k, m * 128:(m + 1) * 128],
                            ctxT[:, k, :], start=(k == 0), stop=(k == 7)
                        )
                    nc.scalar.activation(
                        hidT[:, m, :], ps[:],
                        mybir.ActivationFunctionType.Silu, bias=b1T[:, m:m + 1],
                    )

                # wgen16[n, p*128+d] = hid @ w2p + b2p   (fp16)
                wgen16 = hyp_pool.tile([NB, DO * 9], F16)
                for j in range(3):  # 1152 = 3 * 384
                    ps = tp_psum.tile([NB, 384], F32, tag="wgen")
                    for k in range(4):
                        nc.tensor.matmul(
                            ps[:], hidT[:, k, :],
                            w2p_t[:, k, j * 384:(j + 1) * 384],
                            start=(k == 0), stop=False,
                        )
                    nc.tensor.matmul(
                        ps[:], ones1[:], b2pT[:, j * 384:(j + 1) * 384],
                        start=False, stop=True,
                    )
                    nc.vector.tensor_copy(wgen16[:, j * 384:(j + 1) * 384], ps[:])

                # bias[d] = sum_e s[e] * wb[e, d] + 13 * bb[d]
                bb13 = hyp_pool.tile([128, 1], F32)
                nc.vector.tensor_scalar_mul(bb13[:], bbT[:], float(NB))
                ps_b = tp_psum.tile([128, 1], F32, tag="bias", bufs=1)
                for k in range(8):
                    nc.tensor.matmul(
                        ps_b[:], wbp_t[:, k, :], sT[:, k, :],
                        start=(k == 0), stop=(k == 7)
                    )
                bias_sb = hyp_pool.tile([128, 1], F32)
                nc.scalar.activation(
                    bias_sb[:], ps_b[:],
                    mybir.ActivationFunctionType.Identity, bias=bb13[:],
                )

            # lhsT[dx][n*3+dyi, d] = wgen16[n, (dyi*3+dxi)*128 + d]
            # NOTE: only dim 0 of an SBUF AP crosses partitions, so one DMA
            # per (dx, dy): partition stride 3, offset dyi.
            lhsT = [
                hyp_pool.tile([3 * NB, DO], F16, tag=f"lhsT{i}", name=f"lhsT{i}")
                for i in range(3)
            ]
            wgen16_4d = wgen16[:].rearrange("n (dy dx d) -> n dy dx d", dy=3, dx=3)
            for dxi in range(3):
                lhsT_g = lhsT[dxi][:].rearrange("(n dy) d -> n dy d", dy=3)
                for dyi in range(3):
                    nc.sync.dma_start(
                        lhsT_g[:, dyi, :],
                        wgen16_4d[:, dyi, dxi, :],
                    )

            # ------- phase 0: build the dy-shifted fp16 image in SBUF -------
            # x39[n*3+dyi, r, 1+c] = x[n, r+dy, c]   (zeros at pads / edges)
            x39 = res_pool.tile([3 * NB, HH, WPAD], F16)
            # rows no DMA writes (image edge): zero across all partitions
            # first; the in-range dy groups' DMAs overwrite. Pad columns come
            # from the host-padded source rows.
            nc.gpsimd.memset(x39[:, 0:1, :], 0.0)
            nc.gpsimd.memset(x39[:, HH - 1:HH, :], 0.0)
            x39_g = x39[:].rearrange("(n dy) r w -> n dy r w", dy=3)
            for dyi, dy in enumerate((-1, 0, 1)):
                lo = max(0, -dy)
                hi = min(HH, HH - dy)
                nc.sync.dma_start(
                    x39_g[:, dyi, lo:hi, :],
                    x_ext.ap()[:, lo + dy:hi + dy, :],
                )

            # ---------------- main loop: dynamic conv -----------------------
            NPAIRS = HH // 2                    # 128 two-row pairs
            with (
                tc.tile_pool(name="ostage", bufs=4) as ostage_pool,
                tc.tile_pool(name="cpsum", bufs=GRP, space="PSUM") as cpsum_pool,
            ):
                for _rep in range(repeat):
                    for grp in range(NPAIRS // GRP):
                        psums = [
                            cpsum_pool.tile(
                                [DO, 2, WW], F32, tag="cps", name=f"cps{g}"
                            )
                            for g in range(GRP)
                        ]
                        # dx order (0, -1, +1): the dx=0 matmul reads no pad
                        # columns, keeping its wait count minimal.
                        dx_steps = (1,) if "mm1" in ab else (1, 0, 2)
                        for step, dxi in enumerate(dx_steps):
                            for g in range(GRP):
                                r0 = (grp * GRP + g) * 2
                                nc.tensor.matmul(
                                    psums[g][:],
                                    lhsT[dxi][:],
                                    x39[:, r0:r0 + 2, dxi:dxi + WW],
                                    start=(step == 0),
                                    stop=(step == len(dx_steps) - 1),
                                )
                        for ost_i in range(GRP * 2 // OSTROWS):
                            y0 = grp * GRP * 2 + ost_i * OSTROWS
                            ost = ostage_pool.tile([DO, OSTROWS, WW], F32, tag="ost")
                            for e in range(OSTROWS // 2):
                                g = ost_i * (OSTROWS // 2) + e
                                if g % 2 == 0:
                                    nc.scalar.activation(
                                        ost[:, 2 * e:2 * e + 2, :], psums[g][:],
                                        mybir.ActivationFunctionType.Identity,
                                        bias=bias_sb[:],
                                    )
                                else:
                                    nc.vector.tensor_scalar_add(
                                        ost[:, 2 * e:2 * e + 2, :], psums[g][:],
                                        bias_sb[:],
                                    )
                            # rotate output DMAs across SP ring, ACT ring,
                            # and the gpsimd SWDGE path
                            rot = (2 * grp + ost_i) % 3
                            dma_eng = (nc.sync, nc.scalar, nc.gpsimd)[rot]
                            if "outslim" in ab:
                                dma_eng.dma_start(
                                    out_ext.ap()[:, y0:y0 + OSTROWS, 0:16],
                                    ost[:, :, 0:16],
                                )
                            else:
                                dma_eng.dma_start(
                                    out_ext.ap()[:, y0:y0 + OSTROWS, :], ost[:]
                                )
    if not nc.is_finalized():
        nc.finalize()
    return nc


_NC_CACHE = None


def _get_bass():
    global _NC_CACHE
    if _NC_CACHE is None:
        _NC_CACHE = _build_bass()
    return _NC_CACHE


def _prep_in_maps(inputs):
    x16 = np.asarray(inputs["x"], dtype=np.float32).astype(np.float16)
    x = np.zeros((x16.shape[0], NB, HH, WPAD), np.float16)
    x[:, :, :, 1:WW + 1] = x16
    t_emb = np.ascontiguousarray(np.asarray(inputs["t_emb"], dtype=np.float32))
    wv = np.ascontiguousarray(np.asarray(inputs["wv_embs"], dtype=np.float32))
    w1 = np.asarray(inputs["w1"], dtype=np.float32)
    b1 = np.ascontiguousarray(np.asarray(inputs["b1"], dtype=np.float32))
    w2 = np.asarray(inputs["w2"], dtype=np.float32)
    b2 = np.asarray(inputs["b2"], dtype=np.float32)
    wb = np.asarray(inputs["wb"], dtype=np.float32)
    bb = np.ascontiguousarray(np.asarray(inputs["bb"], dtype=np.float32))

    # permute filter columns: c = d*9 + p  ->  c' = p*128 + d; cast to fp16
    w2p = w2.reshape(4 * DO, DO, 9).transpose(0, 2, 1).reshape(4 * DO, DO * 9)
    w2pp = np.ascontiguousarray(
        w2p.reshape(4, 128, DO * 9).transpose(1, 0, 2)
    ).astype(np.float16)
    b2p = np.ascontiguousarray(b2.reshape(DO, 9).T.reshape(DO * 9)).astype(np.float16)
    w1p = np.ascontiguousarray(
        w1.reshape(8, 128, 4 * DO).transpose(1, 0, 2)
    ).astype(np.float16)
    wbp = np.ascontiguousarray(
        wb.reshape(8, 128, DO).transpose(1, 0, 2)
    ).astype(np.float16)

    return [
        {
            "x": x[b], "t_emb": t_emb[b], "wv": wv[b],
            "w1p": w1p, "b1": b1, "w2pp": w2pp, "b2p": b2p,
            "wbp": wbp, "bb": bb,
        }
        for b in range(NCORES)
    ]


def kernel(**inputs) -> np.ndarray:
    nc = _get_bass()
    in_maps = _prep_in_maps(inputs)
    res = run_bass_kernel_spmd(nc, in_maps, list(range(NCORES)))
    return np.stack([res.results[b]["out"] for b in range(NCORES)], axis=0)


if __name__ == "__main__":
    rng = np.random.default_rng(0)
    demo = {
        "x": rng.standard_normal((NCORES, NB, HH, WW), dtype=np.float32),
        "t_emb": rng.standard_normal((NCORES, DE), dtype=np.float32),
        "wv_embs": rng.standard_normal((NCORES, NB, DE), dtype=np.float32),
        "w1": rng.standard_normal((DE, 4 * DO), dtype=np.float32) * 0.02,
        "b1": np.zeros(4 * DO, np.float32),
        "w2": rng.standard_normal((4 * DO, DO * 9), dtype=np.float32) * 0.02,
        "b2": np.zeros(DO * 9, np.float32),
        "wb": rng.standard_normal((DE, DO), dtype=np.float32) * 0.02,
        "bb": np.zeros(DO, np.float32),
    }
    out = kernel(**demo)
    print("out", out.shape, out.dtype, float(np.abs(out).mean()))



# revision 17
# speedup vs baseline: 1.4680x; 1.4680x over previous
"""Trainium2 Bass kernel for nn_DiffusionDynamicInput.

Reference computation (per sample b):
    ctx  = wv_embs[b] + t_emb[b]                       (13, 1024)
    hid  = silu(ctx @ w1 + b1)                         (13, 512)
    wgen = (hid @ w2 + b2).reshape(13, 128, 9)         per-(band) 3x3 filters
    out[d,h,w] = sum_{n,dy,dx} wgen[n,d,(dy,dx)] * x[b,n,h+dy,w+dx]   (SAME pad)
    bias = (ctx @ wb + bb).sum(axis=0)                 (128,)
    out += bias[:, None, None]

Sharding: data-parallel over B=8 across the 8 NeuronCores (one sample per
core).

Per core the dynamic conv contracts over (band, dy, dx) = 117 terms. The
SBUF image x78 holds the dy-shifted band images twice: partitions 0-38 =
(dy, band) at dx-shift 0, partitions 39-77 = the same shifted one column
left (dx +1 baked in). Rows >= AROWS run 2 matmul passes per psum tile
(78-partition pass covering dx in {0,+1} then a 39-partition pass for
dx=-1 via a free-dim offset); rows < AROWS run the 3-pass 39-partition
schedule and skip the second replica's DMA traffic. AROWS balances PE
time against DMA bytes.

DMA-instruction count and ORDER both matter: each DMA costs ~0.6-1.3 us
of serialized issue (shared HWDGE device + issuing sequencer), and
transfers drain through a serial engine pool, so multi-MB image chunks
must not queue ahead of the small latency-critical transfers (w2 blocks,
lhsT re-layouts). Hence: dy-shifted copies are materialized host-side
into DRAM tensor x3 (edge zero-rows baked in) making each row-chunk ONE
contiguous DMA per image half; hypernetwork inputs arrive as host-packed
blobs with w2 split j-major so wgen can start before the full weight
load; early x chunks interleave with the weight blocks; later x chunks
are issued inside the conv loop (paced prefetch). The output is written
as fp16 (host casts back to fp32; tolerance is 2e-2), halving the
dominant output DMA traffic. PSUM eviction fuses the per-sample bias and
the fp16 cast (ACT + DVE); output DMAs alternate the two HWDGE rings
(SP/ACT); lhsT re-layout DMAs are spread across SP/ACT/Pool, pipelined
with the wgen column blocks.
"""

import numpy as np

import concourse.bacc as bacc
import concourse.mybir as mybir
import concourse.tile as tile
from concourse.bass_utils import run_bass_kernel_spmd

F32 = mybir.dt.float32
F16 = mybir.dt.float16

NB = 13          # bands
HH = WW = 256    # image
DE = 1024        # embed dim
DO = 128         # out channels
NCORES = 8

WPAD = WW + 2    # 258: row layout with a zero column at each end
PSROWS = 4       # rows per psum tile (2 banks; 2 matmuls of 2 rows per pass)
OSTROWS = 16     # output rows per staging tile / output DMA (1 MB DMAs)
AROWS = 96       # rows using the 3-pass/39-partition scheme (rest: 2-pass)

# hypf32 blob columns: [tT (8) | b1T (4) | bbT (1) | wvT (8*13)]
C_T, C_B1, C_BB, C_WV = 0, 8, 12, 13
NF32 = 13 + 8 * NB
# hypf16 blob columns: [w1p (8*512) | w2pp j-major (3*4*384) | wbp (8*128)]
C_W1, C_W2, C_WB = 0, 4096, 8704
NF16 = 8 * 512 + 4 * 1152 + 8 * DO


def _build_bass(repeat: int = 1, ablate: str = "", arows: int = AROWS,
                tile_major: bool = True, evict_rot: int = 1,
                w1_first: bool = False, out_sp: bool = False,
                warm: int = 0, tail_split: bool = True):
    # Bacc (not plain Bass): its finalize() runs generate_event_semaphores,
    # which splits multi-sem waits that TRN2 instruction structs can't hold.
    nc = bacc.Bacc(target_bir_lowering=False, debug=False)

    # x3[n, dyg, r, w] = x_pad[n, r + dyg - 1, w], zero rows at the edges;
    # x_pad is x host-padded to 258-wide rows (zero col at each end).
    x3_ext = nc.declare_dram_parameter("x3", [NB, 3, HH, WPAD], F16, isOutput=False)
    hf32_ext = nc.declare_dram_parameter("hypf32", [128, NF32], F32, isOutput=False)
    hf16_ext = nc.declare_dram_parameter("hypf16", [128, NF16], F16, isOutput=False)
    b2p_ext = nc.declare_dram_parameter("b2p", [DO * 9], F16, isOutput=False)
    out_ext = nc.declare_dram_parameter("out", [DO, HH, WW], F16, isOutput=True)

    # x chunk plan: (lo, hi, half); half 0 = dx0 replica, 1 = dx+1 replica.
    # B-half chunks cover [arows, 256). Only the first 32 rows load up front
    # (before the weight blocks in the transfer queue); the next chunk loads
    # right after the lhsT re-layout DMAs; the rest are emitted inside the
    # conv loop (after each group's output DMA) so their multi-MB transfers
    # never queue ahead of the latency-critical small DMAs.
    assert arows % 32 == 0
    a_bounds = [0, 32, 64, 96, 160, 224, HH]
    b_bounds = sorted({max(arows, c) for c in (0, 32, 64, 96, 160, 224, HH)})
    a_chunks = [(lo, hi, 0) for lo, hi in zip(a_bounds[:-1], a_bounds[1:])]
    b_chunks = [(lo, hi, 1) for lo, hi in zip(b_bounds[:-1], b_bounds[1:])
                if hi > lo]
    up_chunks = sorted(
        [c for c in a_chunks if c[0] < 96] + [c for c in b_chunks if c[0] < 96]
    )
    post_chunks = sorted(
        [c for c in a_chunks if 96 <= c[0] < 160]
        + [c for c in b_chunks if 96 <= c[0] < 160]
    )
    loop_chunks = sorted(
        [c for c in a_chunks if c[0] >= 160] + [c for c in b_chunks if c[0] >= 160]
    )

    with tile.TileContext(nc) as tc:
        with (
            tc.tile_pool(name="const", bufs=1) as const_pool,
            tc.tile_pool(name="resident", bufs=1) as res_pool,
            tc.tile_pool(name="hyp", bufs=1) as hyp_pool,
        ):
            x78 = res_pool.tile([78, HH, WPAD], F16)
            x3ap = x3_ext.ap()

            def load_chunk(eng, chunk):
                lo, hi, half = chunk
                eng.dma_start(
                    x78[half * 39:half * 39 + 39, lo:hi, 0:WPAD - half],
                    x3ap[:, :, lo:hi, half:WPAD],
                )

            # ------------- input DMAs (program order = issue order) ---------
            # SP issue order interleaves image chunks between the weight
            # blocks they must not delay.
            hf32 = hyp_pool.tile([128, NF32], F32)
            hf16 = hyp_pool.tile([128, NF16], F16)
            if w1_first:
                nc.sync.dma_start(hf16[:, 0:C_W2], hf16_ext.ap()[:, 0:C_W2])
                nc.sync.dma_start(hf32[:], hf32_ext.ap())
            else:
                nc.sync.dma_start(hf32[:], hf32_ext.ap())
                nc.sync.dma_start(hf16[:, 0:C_W2], hf16_ext.ap()[:, 0:C_W2])
            nc.sync.dma_start(
                hf16[:, C_W2:C_W2 + 1536], hf16_ext.ap()[:, C_W2:C_W2 + 1536]
            )
            nc.sync.dma_start(
                hf16[:, C_W2 + 1536:C_W2 + 3072],
                hf16_ext.ap()[:, C_W2 + 1536:C_W2 + 3072],
            )
            nc.sync.dma_start(
                hf16[:, C_W2 + 3072:NF16], hf16_ext.ap()[:, C_W2 + 3072:NF16]
            )
            b2pT = hyp_pool.tile([1, DO * 9], F16)
            nc.gpsimd.dma_start(b2pT[:], b2p_ext.ap().rearrange("(o c) -> o c", o=1))
            for c in up_chunks:
                load_chunk(nc.sync, c)
            ones1 = const_pool.tile([1, NB], F16)
            nc.vector.memset(ones1[:], 1.0)
            if warm:
                # keep the PE p-state ramp warming during the DMA prologue
                wrm = const_pool.tile([1, 256], F16)
                nc.vector.memset(wrm[:], 0.0)
                with tc.tile_pool(name="warm_ps", bufs=1, space="PSUM") as wpool:
                    wps = wpool.tile([1, 256], F32)
                    for _ in range(warm):
                        nc.tensor.matmul(wps[:], wrm[:, 0:1], wrm[:],
                                         start=True, stop=True)

            # ---------------- hypernetwork (fp16 in / fp32 psum) ------------
            # ctxT[e, k, n] = wvT[e, k, n] + tT[e, k]   (fp16)
            ctxT = hyp_pool.tile([128, 8, NB], F16)
            for k in range(8):
                nc.vector.tensor_scalar_add(
                    ctxT[:, k, :], hf32[:, C_WV + NB * k:C_WV + NB * (k + 1)],
                    hf32[:, C_T + k:C_T + k + 1],
                )

            # conv lhsT tiles, partition q = half*39 + n*3 + dyg (n-major, so
            # each 39-partition lhsT block is ONE contiguous DMA from wgen16).
            # wgen16 columns are dxi-major: c = dxi*384 + dyg*128 + d.
            lhsT1 = hyp_pool.tile([78, DO], F16, name="lhsT1")
            lhsT2 = hyp_pool.tile([39, DO], F16, name="lhsT2")
            lhsTp1 = hyp_pool.tile([39, DO], F16, name="lhsTp1")

            with tc.tile_pool(name="tp_psum", bufs=2, space="PSUM") as tp_psum:
                # hidT[s, m, n] = silu(sum_e w1[e, m*128+s] * ctxT[e, n] + b1)
                hidT = hyp_pool.tile([128, 4, NB], F16)
                for m in range(4):
                    ps = tp_psum.tile([128, NB], F32, tag="hid")
                    for k in range(8):
                        nc.tensor.matmul(
                            ps[:],
                            hf16[:, C_W1 + m * 1024 + k * 128:
                                 C_W1 + m * 1024 + (k + 1) * 128],
                            ctxT[:, k, :], start=(k == 0), stop=(k == 7)
                        )
                    nc.scalar.activation(
                        hidT[:, m, :], ps[:],
                        mybir.ActivationFunctionType.Silu,
                        bias=hf32[:, C_B1 + m:C_B1 + m + 1],
                    )
                # wgen16[n, dxi*384 + dyg*128 + d] = hid @ w2p + b2p, one
                # j-block per dxi (order: pass-1 weights first); each block's
                # 39-partition lhsT re-layout is ONE DMA, issued on landing.
                wgen16 = hyp_pool.tile([NB, DO * 9], F16)
                for j in (1, 2, 0):
                    ps = tp_psum.tile([NB, 384], F32, tag="wgen")
                    for k in range(4):
                        nc.tensor.matmul(
                            ps[:], hidT[:, k, :],
                            hf16[:, C_W2 + j * 1536 + k * 384:
                                 C_W2 + j * 1536 + (k + 1) * 384],
                            start=(k == 0), stop=False,
                        )
                    nc.tensor.matmul(
                        ps[:], ones1[:], b2pT[:, j * 384:(j + 1) * 384],
                        start=False, stop=True,
                    )
                    nc.vector.tensor_copy(wgen16[:, j * 384:(j + 1) * 384], ps[:])
                    blk = wgen16[:, j * 384:(j + 1) * 384]
                    if j == 1:
                        nc.sync.dma_start(lhsT1[0:39, :], blk)
                    elif j == 2:
                        nc.scalar.dma_start(lhsT1[39:78, :], blk)
                        if arows > 0:
                            nc.gpsimd.dma_start(lhsTp1[:], blk)
                    else:
                        nc.sync.dma_start(lhsT2[:], blk)
                for c in post_chunks:
                    load_chunk(nc.sync, c)

                # bias[d] = sum_e s[e] * wb[e, d] + 13 * bb[d]
                sT32 = hyp_pool.tile([128, 8, 1], F32)
                nc.vector.reduce_sum(sT32[:], ctxT[:], axis=mybir.AxisListType.X)
                sT = hyp_pool.tile([128, 8, 1], F16)
                nc.vector.tensor_copy(sT[:], sT32[:])
                bb13 = hyp_pool.tile([128, 1], F32)
                nc.vector.tensor_scalar_mul(
                    bb13[:], hf32[:, C_BB:C_BB + 1], float(NB)
                )
                ps_b = tp_psum.tile([128, 1], F32, tag="bias", bufs=1)
                for k in range(8):
                    nc.tensor.matmul(
                        ps_b[:],
                        hf16[:, C_WB + k * DO:C_WB + (k + 1) * DO],
                        sT[:, k, :], start=(k == 0), stop=(k == 7)
                    )
                bias_sb = hyp_pool.tile([128, 1], F32)
                nc.scalar.activation(
                    bias_sb[:], ps_b[:],
                    mybir.ActivationFunctionType.Identity, bias=bb13[:],
                )

            # ---------------- main loop: dynamic conv -----------------------
            NGRP = HH // OSTROWS                # 16 groups of 16 rows
            NPS = OSTROWS // PSROWS             # psum tiles per group
            with (
                tc.tile_pool(name="ostage", bufs=4) as ostage_pool,
                tc.tile_pool(name="cpsum", bufs=NPS, space="PSUM") as cpsum_pool,
            ):
                for _rep in range(repeat):
                    pending = list(loop_chunks)
                    for grp in range(NGRP):
                        base = grp * OSTROWS
                        # (lhsT slice, rhs partitions, free-dim col offset)
                        if base < arows:
                            steps = (
                                (lhsT1[0:39, :], 39, 1),
                                (lhsTp1[:], 39, 2),
                                (lhsT2[:], 39, 0),
                            )
                        else:
                            steps = (
                                (lhsT1[:], 78, 1),
                                (lhsT2[:], 39, 0),
                            )
                        psums = [
                            cpsum_pool.tile(
                                [DO, PSROWS, WW], F32, tag="cps", name=f"cps{g}"
                            )
                            for g in range(NPS)
                        ]
                        # per-tile pass-major order: each psum tile finishes
                        # all its passes before the next tile starts, so its
                        # eviction overlaps the remaining tiles' matmuls.
                        if tile_major:
                            for t in range(NPS):
                                for step, (lh, np_, off) in enumerate(steps):
                                    for g2 in range(PSROWS // 2):
                                        r0 = base + t * PSROWS + 2 * g2
                                        nc.tensor.matmul(
                                            psums[t][:, 2 * g2:2 * g2 + 2, :],
                                            lh,
                                            x78[0:np_, r0:r0 + 2, off:off + WW],
                                            start=(step == 0),
                                            stop=(step == len(steps) - 1),
                                        )
                        else:
                            for step, (lh, np_, off) in enumerate(steps):
                                for g in range(OSTROWS // 2):
                                    r0 = base + 2 * g
                                    ps = psums[g // (PSROWS // 2)]
                                    pr = (g % (PSROWS // 2)) * 2
                                    nc.tensor.matmul(
                                        ps[:, pr:pr + 2, :],
                                        lh,
                                        x78[0:np_, r0:r0 + 2, off:off + WW],
                                        start=(step == 0),
                                        stop=(step == len(steps) - 1),
                                    )
                        ost = ostage_pool.tile([DO, OSTROWS, WW], F16, tag="ost")

                        def evict(e):
                            r = e * PSROWS
                            if (e + grp * evict_rot) % 2 == 0:
                                nc.scalar.activation(
                                    ost[:, r:r + PSROWS, :], psums[e][:],
                                    mybir.ActivationFunctionType.Identity,
                                    bias=bias_sb[:],
                                )
                            else:
                                nc.vector.tensor_scalar_add(
                                    ost[:, r:r + PSROWS, :], psums[e][:],
                                    bias_sb[:],
                                )

                        if out_sp:
                            dma_eng, chunk_eng = nc.sync, nc.scalar
                        else:
                            dma_eng = (nc.sync, nc.scalar)[grp % 2]
                            chunk_eng = (nc.scalar, nc.sync)[grp % 2]
                        if tail_split and grp >= NGRP - 2:
                            half = NPS // 2 * PSROWS
                            for e in range(NPS // 2):
                                evict(e)
                            dma_eng.dma_start(
                                out_ext.ap()[:, base:base + half, :],
                                ost[:, 0:half, :],
                            )
                            for e in range(NPS // 2, NPS):
                                evict(e)
                            dma_eng.dma_start(
                                out_ext.ap()[:, base + half:base + OSTROWS, :],
                                ost[:, half:OSTROWS, :],
                            )
                        else:
                            for e in range(NPS):
                                evict(e)
                            dma_eng.dma_start(
                                out_ext.ap()[:, base:base + OSTROWS, :], ost[:]
                            )
                        if pending:
                            load_chunk(chunk_eng, pending.pop(0))
    if not nc.is_finalized():
        nc.finalize()
    return nc


_NC_CACHE = None


def _get_bass():
    global _NC_CACHE
    if _NC_CACHE is None:
        _NC_CACHE = _build_bass()
    return _NC_CACHE


def _prep_in_maps(inputs):
    x16 = np.asarray(inputs["x"], dtype=np.float32).astype(np.float16)
    nb_total = x16.shape[0]
    xpad = np.zeros((nb_total, NB, HH, WPAD), np.float16)
    xpad[:, :, :, 1:WW + 1] = x16
    # x3[b, n, dyg, r, :] = xpad[b, n, r + dyg - 1, :] (zero rows at edges)
    x3 = np.zeros((nb_total, NB, 3, HH, WPAD), np.float16)
    x3[:, :, 0, 1:, :] = xpad[:, :, :HH - 1, :]
    x3[:, :, 1] = xpad
    x3[:, :, 2, :HH - 1, :] = xpad[:, :, 1:, :]

    t_emb = np.asarray(inputs["t_emb"], dtype=np.float32)
    wv = np.asarray(inputs["wv_embs"], dtype=np.float32)
    w1 = np.asarray(inputs["w1"], dtype=np.float32)
    b1 = np.asarray(inputs["b1"], dtype=np.float32)
    w2 = np.asarray(inputs["w2"], dtype=np.float32)
    b2 = np.asarray(inputs["b2"], dtype=np.float32)
    wb = np.asarray(inputs["wb"], dtype=np.float32)
    bb = np.asarray(inputs["bb"], dtype=np.float32)

    # hypf32 blob: [tT | b1T | bbT | wvT]
    hypf32 = np.empty((nb_total, 128, NF32), np.float32)
    hypf32[:, :, C_T:C_B1] = t_emb.reshape(-1, 8, 128).transpose(0, 2, 1)
    hypf32[:, :, C_B1:C_BB] = b1.reshape(4, 128).T
    hypf32[:, :, C_BB] = bb
    hypf32[:, :, C_WV:] = wv.reshape(-1, NB, 8, 128).transpose(0, 3, 2, 1).reshape(
        nb_total, 128, 8 * NB
    )

    # hypf16 blob: [w1p | w2pp (dxi-major j blocks) | wbp], fp16
    # w2p columns: c = dxi*384 + dyg*128 + d
    w2p = (
        w2.reshape(4 * DO, DO, 3, 3)          # (s, d, dyg, dxi)
        .transpose(0, 3, 2, 1)                # (s, dxi, dyg, d)
        .reshape(4 * DO, DO * 9)
    )
    hypf16 = np.empty((128, NF16), np.float16)
    hypf16[:, C_W1:C_W2] = (
        w1.reshape(8, 128, 4, 128)            # (k, p, m, s)
        .transpose(1, 2, 0, 3)                # (p, m, k, s)
        .reshape(128, 8 * 4 * DO)
    )
    hypf16[:, C_W2:C_WB] = (
        w2p.reshape(4, 128, 3, 384)           # (k, p, j, 384)
        .transpose(1, 2, 0, 3)                # (p, j, k, 384)
        .reshape(128, 4 * DO * 9)
    )
    hypf16[:, C_WB:] = wb.reshape(8, 128, DO).transpose(1, 0, 2).reshape(
        128, 8 * DO
    )
    b2p = np.ascontiguousarray(
        b2.reshape(DO, 3, 3).transpose(2, 1, 0).reshape(DO * 9)
    ).astype(np.float16)

    return [
        {
            "x3": x3[b], "hypf32": np.ascontiguousarray(hypf32[b]),
            "hypf16": hypf16, "b2p": b2p,
        }
        for b in range(NCORES)
    ]


def kernel(**inputs) -> np.ndarray:
    nc = _get_bass()
    in_maps = _prep_in_maps(inputs)
    res = run_bass_kernel_spmd(nc, in_maps, list(range(NCORES)))
    return np.stack(
        [res.results[b]["out"].astype(np.float32) for b in range(NCORES)], axis=0
    )


if __name__ == "__main__":
    rng = np.random.default_rng(0)
    demo = {
        "x": rng.standard_normal((NCORES, NB, HH, WW), dtype=np.float32),
        "t_emb": rng.standard_normal((NCORES, DE), dtype=np.float32),
        "wv_embs": rng.standard_normal((NCORES, NB, DE), dtype=np.float32),
        "w1": rng.standard_normal((DE, 4 * DO), dtype=np.float32) * 0.02,
        "b1": np.zeros(4 * DO, np.float32),
        "w2": rng.standard_normal((DE // 2, DO * 9), dtype=np.float32) * 0.02,
        "b2": np.zeros(DO * 9, np.float32),
        "wb": rng.standard_normal((DE, DO), dtype=np.float32) * 0.02,
        "bb": np.zeros(DO, np.float32),
    }
    out = kernel(**demo)
    print("out", out.shape, out.dtype, float(np.abs(out).mean()))
